# revision 2
# baseline (speedup 1.0000x reference)
"""Tile-parallel 2D Gaussian-splat compositor for Trainium2 (8 NeuronCores).

Strategy
--------
Pixels are sharded across 8 cores as horizontal strips (24 rows each).
Within a core the strip is split into 24x16-pixel tiles (F=384 pixels,
free axis); gaussians go on the partition axis in depth-sorted blocks of
128.  Per (tile, block):

  sigma' = G^T @ feat          (PE, K=6 quadratic-form features, fp32)
  alpha  = exp(-sigma')        (ACT; opacity folded into G's const term)
  am     = alpha * (alpha>=1/255)   (DVE scalar_tensor_tensor, 1 op)
  lg     = ln(1 - am)          (ACT)
  S     += strictU^T @ lg      (PE: cross-partition exclusive cumsum)
  T      = exp(S)              (ACT: per-gaussian transmittance)
  w      = T * am              (DVE)
  rgb   += colors^T @ w        (PE: [3,F] accumulated in PSUM)

Host-side: depth sort, conservative per-gaussian bbox cull per tile
(exact: culled pairs provably have alpha < 1/255 -> zero in the
reference too), quadratic-form coefficients in float64, padding with
inert dummy gaussians so all 8 cores run one SPMD program.
"""

import sys

if "/opt/trn_rl_repo" not in sys.path:
    sys.path.insert(0, "/opt/trn_rl_repo")

import numpy as np

H = 192
W = 192
NDEV = 8
STRIP = H // NDEV            # 24 rows per core
TILE_R = 24                  # tile height == strip height
TILE_C = 16                  # tile width
NT = W // TILE_C             # 12 tiles per core
F = TILE_R * TILE_C          # 384 pixels per tile (matmul free dim)
BLK = 128                    # gaussians per block (partition dim)
ALPHA_MIN = 1.0 / 255.0
ALPHA_MAX = 0.999
DUMMY_SIG = 60.0             # sigma' for padding slots -> alpha ~ 0


def _host_prep(means2d, conics, colors, opacities, depths, background):
    """Sort, cull, and pack per-core parameter arrays (all in float64)."""
    m = np.asarray(means2d, np.float64)
    q = np.asarray(conics, np.float64)
    col = np.asarray(colors, np.float64)
    op = np.asarray(opacities, np.float64)
    dep = np.asarray(depths, np.float64)

    order = np.argsort(dep, kind="stable")
    m = m[order]
    q = q[order]
    col = col[order]
    op = op[order]

    mx, my = m[:, 0], m[:, 1]
    A, B, C = q[:, 0], q[:, 1], q[:, 2]

    with np.errstate(divide="ignore", invalid="ignore"):
        tau = np.log(255.0 * op)
        detq = A * C - B * B
        sxx = C / detq
        syy = A / detq
        ex = np.sqrt(np.maximum(2.0 * tau * sxx, 0.0)) * 1.0001 + 1e-3
        ey = np.sqrt(np.maximum(2.0 * tau * syy, 0.0)) * 1.0001 + 1e-3
    valid = (tau > 0) & (detq > 0) & np.isfinite(ex) & np.isfinite(ey)

    eps = 1e-6
    # gaussian index lists per (device, tile), depth order preserved
    idx = [[None] * NT for _ in range(NDEV)]
    cnt = np.zeros((NDEV, NT), np.int64)
    for d in range(NDEV):
        r0 = d * STRIP
        ymask = valid & (my + ey >= r0 + 0.5 - eps) & (my - ey <= r0 + STRIP - 0.5 + eps)
        for t in range(NT):
            c0 = t * TILE_C
            mask = ymask & (mx + ex >= c0 + 0.5 - eps) & (mx - ex <= c0 + TILE_C - 0.5 + eps)
            g = np.nonzero(mask)[0]
            idx[d][t] = g
            cnt[d, t] = len(g)

    nblk = np.maximum(1, -(-cnt.max(axis=0) // BLK))     # [NT] blocks per tile
    off = np.concatenate([[0], np.cumsum(nblk)])         # [NT+1]
    tot = int(off[-1])

    lnop = np.log(op)
    gts, colss = [], []
    for d in range(NDEV):
        r0 = d * STRIP
        gt = np.zeros((6, tot * BLK), np.float64)
        gt[5, :] = DUMMY_SIG
        cl = np.zeros((BLK, tot * 3), np.float64)
        for t in range(NT):
            g = idx[d][t]
            n = len(g)
            if n == 0:
                continue
            c0 = t * TILE_C
            slot = off[t] * BLK + np.arange(n)
            mlx = mx[g] - (c0 + TILE_C / 2.0)
            mly = my[g] - (r0 + TILE_R / 2.0)
            a, b, c = A[g], B[g], C[g]
            gt[0, slot] = 0.5 * a
            gt[1, slot] = 0.5 * c
            gt[2, slot] = b
            gt[3, slot] = -(a * mlx + b * mly)
            gt[4, slot] = -(c * mly + b * mlx)
            gt[5, slot] = 0.5 * a * mlx**2 + 0.5 * c * mly**2 + b * mlx * mly - lnop[g]
            blk_i = off[t] + np.arange(n) // BLK
            part = np.arange(n) % BLK
            cl[part, blk_i * 3 + 0] = col[g, 0]
            cl[part, blk_i * 3 + 1] = col[g, 1]
            cl[part, blk_i * 3 + 2] = col[g, 2]
        gts.append(gt.astype(np.float32))
        colss.append(cl.astype(np.float32))

    # pixel features in tile-local coords (identical for every tile)
    xs = np.arange(TILE_C) + 0.5 - TILE_C / 2.0
    ys = np.arange(TILE_R) + 0.5 - TILE_R / 2.0
    Y, X = np.meshgrid(ys, xs, indexing="ij")
    x, y = X.ravel(), Y.ravel()
    feat = np.stack([x * x, y * y, x * y, x, y, np.ones(F)]).astype(np.float32)

    strict_u = np.triu(np.ones((BLK, BLK), np.float32), 1)   # [k,n]=1 iff k<n
    compl_u = np.tril(np.ones((BLK, BLK), np.float32), 0)    # [k,n]=1 iff k>=n

    return nblk, off, tot, gts, colss, feat, strict_u, compl_u


def _build_program(nblk, tot, bg_nonzero, clamp_alpha):
    import concourse.tile as tile
    import concourse.mybir as mybir
    from concourse import bacc
    from contextlib import ExitStack

    f32 = mybir.dt.float32
    Act = mybir.ActivationFunctionType
    Alu = mybir.AluOpType

    nc = bacc.Bacc("TRN2", target_bir_lowering=False, debug=False)
    feat_d = nc.dram_tensor("feat", [6, F], f32, kind="ExternalInput")
    ut_d = nc.dram_tensor("ut", [BLK, BLK], f32, kind="ExternalInput")
    gt_d = nc.dram_tensor("gt", [6, tot * BLK], f32, kind="ExternalInput")
    cols_d = nc.dram_tensor("cols", [BLK, tot * 3], f32, kind="ExternalInput")
    need_compl = bg_nonzero or any(b > 1 for b in nblk)
    if need_compl:
        cu_d = nc.dram_tensor("cu", [BLK, BLK], f32, kind="ExternalInput")
    if bg_nonzero:
        bg_d = nc.dram_tensor("bg", [1, 3], f32, kind="ExternalInput")
    out_d = nc.dram_tensor("out", [3, STRIP, W], f32, kind="ExternalOutput")

    with tile.TileContext(nc) as tc, ExitStack() as ctx:
        cpool = ctx.enter_context(tc.tile_pool(name="consts", bufs=1))
        sb = ctx.enter_context(tc.tile_pool(name="sb", bufs=3))
        ps_sig = ctx.enter_context(tc.tile_pool(name="ps_sig", bufs=2, space="PSUM"))
        ps_s = ctx.enter_context(tc.tile_pool(name="ps_s", bufs=2, space="PSUM"))
        ps_col = ctx.enter_context(tc.tile_pool(name="ps_col", bufs=2, space="PSUM"))

        feat = cpool.tile([6, F], f32)
        nc.sync.dma_start(feat[:], feat_d.ap())
        ut = cpool.tile([BLK, BLK], f32)
        nc.sync.dma_start(ut[:], ut_d.ap())
        gt = cpool.tile([6, tot * BLK], f32)
        nc.sync.dma_start(gt[:], gt_d.ap())
        cols = cpool.tile([BLK, tot * 3], f32)
        nc.sync.dma_start(cols[:], cols_d.ap())
        if need_compl:
            cu = cpool.tile([BLK, BLK], f32)
            nc.sync.dma_start(cu[:], cu_d.ap())
        if bg_nonzero:
            bgt = cpool.tile([1, 3], f32)
            nc.sync.dma_start(bgt[:], bg_d.ap())

        out_ap = out_d.ap()
        for t in range(NT):
            bt = int(nblk[t])
            off_t = int(np.sum(nblk[:t]))
            s_ps = ps_s.tile([BLK, F], f32)
            colp = ps_col.tile([3, F], f32)
            for b in range(bt):
                blk = off_t + b
                sig = ps_sig.tile([BLK, F], f32)
                nc.tensor.matmul(
                    sig[:], gt[:, blk * BLK:(blk + 1) * BLK], feat[:],
                    start=True, stop=True,
                )
                alpha = sb.tile([BLK, F], f32)
                nc.scalar.activation(alpha[:], sig[:], Act.Exp, scale=-1.0)
                if clamp_alpha:
                    nc.vector.tensor_scalar_min(alpha[:], alpha[:], ALPHA_MAX)
                am = sb.tile([BLK, F], f32)
                nc.vector.scalar_tensor_tensor(
                    am[:], alpha[:], ALPHA_MIN, alpha[:],
                    op0=Alu.is_ge, op1=Alu.mult,
                )
                lg = sb.tile([BLK, F], f32)
                nc.scalar.activation(lg[:], am[:], Act.Ln, bias=1.0, scale=-1.0)
                nc.tensor.matmul(
                    s_ps[:], ut[:], lg[:],
                    start=(b == 0), stop=(b == bt - 1 and not need_compl),
                    skip_group_check=True,
                )
                tr = sb.tile([BLK, F], f32)
                nc.scalar.activation(tr[:], s_ps[:], Act.Exp)
                w = sb.tile([BLK, F], f32)
                nc.vector.tensor_mul(w[:], tr[:], am[:])
                nc.tensor.matmul(
                    colp[:], cols[:, blk * 3:(blk + 1) * 3], w[:],
                    start=(b == 0), stop=(b == bt - 1 and not bg_nonzero),
                    skip_group_check=True,
                )
                if need_compl and (b < bt - 1 or bg_nonzero):
                    nc.tensor.matmul(
                        s_ps[:], cu[:], lg[:],
                        start=False, stop=(b == bt - 1),
                        skip_group_check=True,
                    )
            if bg_nonzero:
                tfin = sb.tile([1, F], f32)
                nc.scalar.activation(tfin[:], s_ps[0:1, :], Act.Exp)
                nc.tensor.matmul(
                    colp[:], bgt[:], tfin[:],
                    start=False, stop=True, skip_group_check=True,
                )
            ostage = sb.tile([3, F], f32)
            nc.vector.tensor_copy(ostage[:], colp[:])
            nc.sync.dma_start(
                out_ap[:, :, t * TILE_C:(t + 1) * TILE_C],
                ostage[:].rearrange("c (h w) -> c h w", h=TILE_R),
            )
    nc.compile()
    return nc


def kernel(means2d, conics, colors, opacities, depths, background):
    from concourse import bass_utils

    nblk, off, tot, gts, colss, feat, strict_u, compl_u = _host_prep(
        means2d, conics, colors, opacities, depths, background
    )
    bg = np.asarray(background, np.float32)
    bg_nonzero = bool(np.any(bg != 0))
    clamp_alpha = bool(np.asarray(opacities).max() >= ALPHA_MAX)

    nc = _build_program(nblk, tot, bg_nonzero, clamp_alpha)

    in_maps = []
    for d in range(NDEV):
        im = {"feat": feat, "ut": strict_u, "gt": gts[d], "cols": colss[d]}
        if bg_nonzero or any(b > 1 for b in nblk):
            im["cu"] = compl_u
        if bg_nonzero:
            im["bg"] = bg.reshape(1, 3)
        in_maps.append(im)

    res = bass_utils.run_bass_kernel_spmd(nc, in_maps, core_ids=list(range(NDEV)))
    img = np.concatenate([res.results[d]["out"] for d in range(NDEV)], axis=1)
    return img.astype(np.float32)


if __name__ == "__main__":
    import reference

    inputs = {k: np.asarray(v) for k, v in reference.setup_inputs().items()}
    out = kernel(**inputs)
    print("kernel output:", out.shape, out.dtype)
